# revision 11
# baseline (speedup 1.0000x reference)
"""Multi-head attention (qk-norm variant) on 8 TRN2 NeuronCores.

Sharding (Megatron-style, per spec hint): core c handles batch b=c//4 and
head-group hg=c%4 (4 of 16 heads). QKV is column-parallel (each core owns its
heads' rows of w_qkv), attention is fully local per (b, head), and the output
projection is row-parallel: each core produces a partial [N, DIM] f32 output
which the host sums per batch (the "unshard" step) and adds b_proj.

Per-core kernel (bf16 compute, fp32 PSUM accumulation):
  - x arrives pre-transposed (xT [DIM, N]) so the feature dim lies on SBUF
    partitions for all matmuls.
  - q,k are produced head-major ([d, tok], 2 heads stacked per 128
    partitions) for the scores matmul; layernorm over head_dim (the
    partition dim) is done with matmuls: centering via C2 = blockdiag(I-J/64)
    and sum-of-squares via ones-column matmuls; the per-token rstd rows are
    broadcast across partitions with gpsimd.partition_broadcast (base-0
    tiles; the consuming DVE muls use shifted partition bases).
  - softmax needs no max-subtraction: after qk-norm, |q|=|k|=8 exactly, so
    scores are in [-8, 8] and exp() is safe.
  - v is token-major with a fused ones-column, so the PV matmul produces the
    softmax denominator as psum row 64 for free; normalization is a rank-1
    scale applied after PV (reciprocal_approx_fast + partition_broadcast).
"""
import numpy as np
import ml_dtypes

import concourse.bacc as bacc
import concourse.tile as tile
from concourse import mybir
from concourse.bass_utils import run_bass_kernel_spmd

F32 = mybir.dt.float32
BF16 = mybir.dt.bfloat16
AF = mybir.ActivationFunctionType

B, N, DIM = 2, 2048, 1024
H, D = 16, 64
EPS = 1e-5
N_CORES = 8
HPC = 4              # heads per core
HF = HPC * D         # 256 local head features
KT = DIM // 128      # 8 contraction tiles
NT = N // 128        # 16 token tiles
NCH = N // 512       # 4 token chunks
SCALE = D ** -0.5

# set by test harness to request NTFF profiling
TRACE = False
LAST_EXEC_NS = None
LAST_RESULTS = None

_BUILD_CACHE = {}


def _build(has_qgamma, has_kgamma, has_qbeta, has_kbeta, has_vbias):
    key = (has_qgamma, has_kgamma, has_qbeta, has_kbeta, has_vbias)
    if key in _BUILD_CACHE:
        return _BUILD_CACHE[key]

    nc = bacc.Bacc("TRN2", target_bir_lowering=False, debug=False,
                   num_devices=N_CORES)

    xT_d = nc.dram_tensor("xT", [DIM, N], BF16, kind="ExternalInput")
    wqkT_d = nc.dram_tensor("wqkT", [DIM, 2 * HF], BF16, kind="ExternalInput")
    wvT_d = nc.dram_tensor("wvT", [DIM, HF], BF16, kind="ExternalInput")
    wpT_d = nc.dram_tensor("wpT", [HF, DIM], BF16, kind="ExternalInput")
    bqk_d = nc.dram_tensor("bqk_cols", [128, 4], F32, kind="ExternalInput")
    bvT_d = nc.dram_tensor("bvT", [1, HF], BF16, kind="ExternalInput")
    C2_d = nc.dram_tensor("C2", [128, 128], BF16, kind="ExternalInput")
    O2_d = nc.dram_tensor("O2", [128, 2], BF16, kind="ExternalInput")
    ones_d = nc.dram_tensor("ones512", [1, 512], BF16, kind="ExternalInput")
    gamma_d = beta_d = None
    if has_qgamma or has_kgamma:
        gamma_d = nc.dram_tensor("gamma_cols", [128, 2], F32, kind="ExternalInput")
    if has_qbeta or has_kbeta:
        beta_d = nc.dram_tensor("beta_cols", [128, 2], F32, kind="ExternalInput")
    out_d = nc.dram_tensor("out_partial", [N, DIM], F32, kind="ExternalOutput")

    with tile.TileContext(nc) as tc:
        with (
            tc.tile_pool(name="persist", bufs=1) as pp,
            tc.tile_pool(name="work", bufs=2) as wp,
        ):
            # ---- persistent SBUF tensors ----
            xT = [pp.tile([128, N], BF16, name=f"xT{i}") for i in range(KT)]
            wqk = [pp.tile([128, 2 * HF], BF16, name=f"wqk{i}") for i in range(KT)]
            wv = [pp.tile([128, HF], BF16, name=f"wv{i}") for i in range(KT)]
            wpj = [pp.tile([128, DIM], BF16, name=f"wpj{i}") for i in range(2)]
            C2 = pp.tile([128, 128], BF16)
            O2 = pp.tile([128, 2], BF16)
            ones512 = pp.tile([1, 512], BF16)
            bqk = pp.tile([128, 4], F32)
            bvT = pp.tile([1, HF], BF16)
            eps_sb = pp.tile([1, 1], F32)
            gamma_c = pp.tile([128, 2], F32) if gamma_d is not None else None
            beta_c = pp.tile([128, 2], F32) if beta_d is not None else None

            # v token-major with a ones column at index 64 (width 66 keeps the
            # innermost dim even for DVE perf modes)
            v_sb = pp.tile([128, NT, HPC, 66], BF16)
            # qk_raw is overwritten in place with the centered values after
            # the center matmul (Tile serializes the WAR hazard)
            qk_raw = pp.tile([128, 4, N], BF16)   # [., g, tok] g: q01,q23,k01,k23
            qn = pp.tile([128, 4, N], BF16)       # normalized
            outT_n = pp.tile([128, 2, N], BF16)   # attn out, head-major

            for i in range(KT):
                nc.sync.dma_start(out=xT[i], in_=xT_d.ap()[i * 128:(i + 1) * 128, :])
                nc.sync.dma_start(out=wqk[i], in_=wqkT_d.ap()[i * 128:(i + 1) * 128, :])
                nc.sync.dma_start(out=wv[i], in_=wvT_d.ap()[i * 128:(i + 1) * 128, :])
            for i in range(2):
                nc.sync.dma_start(out=wpj[i], in_=wpT_d.ap()[i * 128:(i + 1) * 128, :])
            for t, d in [(C2, C2_d), (O2, O2_d), (ones512, ones_d),
                         (bqk, bqk_d), (bvT, bvT_d)]:
                nc.sync.dma_start(out=t, in_=d.ap())
            if gamma_c is not None:
                nc.sync.dma_start(out=gamma_c, in_=gamma_d.ap())
            if beta_c is not None:
                nc.sync.dma_start(out=beta_c, in_=beta_d.ap())

            nc.vector.memset(eps_sb, EPS)
            nc.vector.memset(v_sb[:, :, :, 64:66], 0.0)
            nc.vector.memset(v_sb[:, :, :, 64:65], 1.0)

            with tc.tile_pool(name="ps1", bufs=1, space="PSUM") as ps1:
                # ---- phase B: qkv projections ----
                with nc.named_scope("qkv"):
                    # q,k head-major: psum[feat, tok] = wqk_tile.T @ xT
                    for mt in range(4):
                        for ch in range(NCH):
                            csl = slice(ch * 512, (ch + 1) * 512)
                            ps_qk = ps1.tile([128, 512], F32, tag="big", bufs=4)
                            for kt in range(KT):
                                nc.tensor.matmul(
                                    ps_qk,
                                    wqk[kt][:, mt * 128:(mt + 1) * 128],
                                    xT[kt][:, csl],
                                    start=(kt == 0), stop=(kt == KT - 1))
                            # fold the qkv bias (per-feature = per-partition)
                            nc.vector.tensor_scalar_add(
                                qk_raw[:, mt, csl], ps_qk, bqk[:, mt:mt + 1])

                    # v token-major: psum[tok, vfeat] = xT_tile.T @ wvT
                    for tt in range(NT):
                        tsl = slice(tt * 128, (tt + 1) * 128)
                        ps_v = ps1.tile([128, 512], F32, tag="big", bufs=4)
                        for kt in range(KT):
                            nc.tensor.matmul(
                                ps_v[:, 0:HF], xT[kt][:, tsl], wv[kt],
                                start=(kt == 0),
                                stop=(not has_vbias and kt == KT - 1))
                        if has_vbias:
                            # bias via K=1 matmul: ones[tok] x bvT[feat]
                            nc.tensor.matmul(ps_v[:, 0:HF], ones512[:, 0:128],
                                             bvT, start=False, stop=True)
                        nc.vector.tensor_copy(
                            v_sb[:, tt, :, 0:64],
                            ps_v[:, 0:HF].rearrange("p (h d) -> p h d", h=HPC))

                # ---- phase C: qk layernorm over head_dim (partition axis) ----
                with nc.named_scope("ln"):
                    # group order q01, k01, q23, k23 lets heads 0/1 start early
                    for g in (0, 2, 1, 3):
                        is_q = g < 2
                        gcol = None
                        if is_q and has_qgamma:
                            gcol = gamma_c[:, 0:1]
                        elif not is_q and has_kgamma:
                            gcol = gamma_c[:, 1:2]
                        bcol = None
                        if is_q and has_qbeta:
                            bcol = beta_c[:, 0:1]
                        elif not is_q and has_kbeta:
                            bcol = beta_c[:, 1:2]
                        for ch in range(NCH):
                            csl = slice(ch * 512, (ch + 1) * 512)
                            ps_c = ps1.tile([128, 512], F32, tag="big", bufs=4)
                            nc.tensor.matmul(ps_c, C2, qk_raw[:, g, csl],
                                             start=True, stop=True)
                            nc.vector.tensor_copy(qk_raw[:, g, csl], ps_c)
                            sq = wp.tile([128, 512], BF16, tag="sq")
                            nc.vector.tensor_mul(sq, qk_raw[:, g, csl],
                                                 qk_raw[:, g, csl])
                            ps_ssq = ps1.tile([1, 2, 512], F32, tag="small", bufs=2)
                            for j in range(2):
                                nc.tensor.matmul(ps_ssq[:, j, :], O2[:, j:j + 1],
                                                 sq, start=True, stop=True)
                            std = wp.tile([1, 2, 512], F32, tag="std")
                            nc.scalar.activation(std, ps_ssq, AF.Sqrt,
                                                 scale=1.0 / D, bias=eps_sb)
                            rstd = wp.tile([1, 2, 512], F32, tag="rstd")
                            nc.vector.reciprocal_approx_fast(rstd, std)
                            rb_a = wp.tile([64, 512], F32, tag="rb_a")
                            rb_b = wp.tile([64, 512], F32, tag="rb_b")
                            nc.gpsimd.partition_broadcast(rb_a, rstd[:, 0, :])
                            nc.gpsimd.partition_broadcast(rb_b, rstd[:, 1, :])
                            # in0 from PSUM: walrus requires equal base
                            # partitions only when BOTH TT inputs are SBUF
                            nc.vector.tensor_mul(qn[0:64, g, csl],
                                                 ps_c[0:64, :], rb_a)
                            nc.vector.tensor_mul(qn[64:128, g, csl],
                                                 ps_c[64:128, :], rb_b)
                            if gcol is not None:
                                nc.vector.tensor_scalar_mul(
                                    qn[:, g, csl], qn[:, g, csl], gcol)
                            if bcol is not None:
                                nc.vector.tensor_scalar_add(
                                    qn[:, g, csl], qn[:, g, csl], bcol)

            with (
                tc.tile_pool(name="ps2", bufs=1, space="PSUM") as ps2,
                tc.tile_pool(name="ps3", bufs=1, space="PSUM") as ps3,
            ):
                # ---- phase D: attention (4 heads) + interleaved projection ----
                with nc.named_scope("attn"):
                    for qc_i in range(NCH):
                        qsl = slice(qc_i * 512, (qc_i + 1) * 512)
                        exp_ts = []
                        # scores + exp for all heads first (keeps PE dense
                        # while ACT chews through the exps). The two heads of
                        # a pair sit at base partitions 0/64, so interleaving
                        # their matmuls packs them into disjoint PE row
                        # groups (concurrent execution).
                        for h in range(HPC):
                            exp_ts.append(wp.tile([128, NT, 512], BF16,
                                                  tag="exp", bufs=4,
                                                  name=f"exp{h}"))
                        for gq in range(2):
                            for k2 in range(NT // 2):
                                ps_pair = []
                                for hp in range(2):
                                    p0 = hp * 64
                                    ps_s = ps2.tile([128, 2, 512], F32,
                                                    tag="score", bufs=2,
                                                    name=f"ps_s{hp}")
                                    ps_pair.append(ps_s)
                                    for j in range(2):
                                        kt = k2 * 2 + j
                                        nc.tensor.matmul(
                                            ps_s[:, j, :],
                                            qn[p0:p0 + 64, 2 + gq,
                                               kt * 128:(kt + 1) * 128],
                                            qn[p0:p0 + 64, gq, qsl],
                                            start=True, stop=True)
                                for hp in range(2):
                                    nc.scalar.activation(
                                        exp_ts[2 * gq + hp][:, k2 * 2:k2 * 2 + 2, :],
                                        ps_pair[hp], AF.Exp, scale=SCALE)
                        # PV + normalize per head
                        for h in range(HPC):
                            gq, p0 = h // 2, (h % 2) * 64
                            exp_t = exp_ts[h]
                            ps_o = ps2.tile([65, 512], F32, tag="pv", bufs=2)
                            for kt in range(NT):
                                nc.tensor.matmul(ps_o, v_sb[:, kt, h, 0:65],
                                                 exp_t[:, kt, :],
                                                 start=(kt == 0),
                                                 stop=(kt == NT - 1))
                            # reciprocal_approx_fast misreads PSUM sources —
                            # stage the denominator row through SBUF first
                            den = wp.tile([1, 512], F32, tag="den")
                            nc.vector.tensor_copy(den, ps_o[64:65, :])
                            rec = wp.tile([1, 512], F32, tag="rec")
                            nc.vector.reciprocal_approx_fast(rec, den)
                            rb2 = wp.tile([64, 512], F32, tag="rb2")
                            nc.gpsimd.partition_broadcast(rb2, rec)
                            nc.vector.tensor_mul(outT_n[p0:p0 + 64, gq, qsl],
                                                 ps_o[0:64, :], rb2)

                        # projection for this chunk's 4 token tiles
                        with nc.named_scope("proj"):
                            for tt in range(qc_i * 4, qc_i * 4 + 4):
                                tsl = slice(tt * 128, (tt + 1) * 128)
                                for fn in range(2):
                                    fsl = slice(fn * 512, (fn + 1) * 512)
                                    ps_p = ps3.tile([128, 512], F32, tag="pj",
                                                    bufs=2)
                                    for t in range(2):
                                        nc.tensor.matmul(ps_p,
                                                         outT_n[:, t, tsl],
                                                         wpj[t][:, fsl],
                                                         start=(t == 0),
                                                         stop=(t == 1))
                                    ostg = wp.tile([128, 512], F32, tag="ostg",
                                                   bufs=3)
                                    nc.vector.tensor_copy(ostg, ps_p)
                                    nc.sync.dma_start(out=out_d.ap()[tsl, fsl],
                                                      in_=ostg)

    nc.compile()
    _BUILD_CACHE[key] = nc
    return nc


def _bf16(a):
    return np.ascontiguousarray(a).astype(ml_dtypes.bfloat16)


def kernel(**inputs):
    global LAST_EXEC_NS
    x = np.asarray(inputs["x"], np.float32)
    w_qkv = np.asarray(inputs["w_qkv"], np.float32)
    b_qkv = np.asarray(inputs["b_qkv"], np.float32)
    q_gamma = np.asarray(inputs["q_gamma"], np.float32)
    q_beta = np.asarray(inputs["q_beta"], np.float32)
    k_gamma = np.asarray(inputs["k_gamma"], np.float32)
    k_beta = np.asarray(inputs["k_beta"], np.float32)
    w_proj = np.asarray(inputs["w_proj"], np.float32)
    b_proj = np.asarray(inputs["b_proj"], np.float32)

    has_qgamma = not bool(np.all(q_gamma == 1.0))
    has_kgamma = not bool(np.all(k_gamma == 1.0))
    has_qbeta = bool(np.any(q_beta != 0.0))
    has_kbeta = bool(np.any(k_beta != 0.0))
    has_vbias = bool(np.any(b_qkv[2 * DIM:3 * DIM] != 0.0))
    nc = _build(has_qgamma, has_kgamma, has_qbeta, has_kbeta, has_vbias)

    # shared constants
    Cd = np.eye(D, dtype=np.float32) - 1.0 / D
    C2 = np.zeros((128, 128), np.float32)
    C2[:D, :D] = Cd
    C2[D:, D:] = Cd
    O2 = np.zeros((128, 2), np.float32)
    O2[:D, 0] = 1.0
    O2[D:, 1] = 1.0
    ones512 = np.ones((1, 512), np.float32)
    gamma_cols = np.stack([np.tile(q_gamma, 2), np.tile(k_gamma, 2)],
                          axis=1).astype(np.float32)
    beta_cols = np.stack([np.tile(q_beta, 2), np.tile(k_beta, 2)],
                         axis=1).astype(np.float32)

    in_maps = []
    for c in range(N_CORES):
        b, hg = divmod(c, 4)
        rows = slice(hg * HF, (hg + 1) * HF)
        q_l = w_qkv[0 * DIM:1 * DIM][rows]           # [256, 1024]
        k_l = w_qkv[1 * DIM:2 * DIM][rows]
        v_l = w_qkv[2 * DIM:3 * DIM][rows]
        bq_l = b_qkv[0 * DIM:1 * DIM][rows]
        bk_l = b_qkv[1 * DIM:2 * DIM][rows]
        bv_l = b_qkv[2 * DIM:3 * DIM][rows]
        bqk_cols = np.stack([bq_l[:128], bq_l[128:], bk_l[:128], bk_l[128:]],
                            axis=1).astype(np.float32)
        m = {
            "xT": _bf16(x[b].T),                          # [1024, 2048]
            "wqkT": _bf16(np.concatenate([q_l, k_l], 0).T),   # [1024, 512]
            "wvT": _bf16(v_l.T),                          # [1024, 256]
            "wpT": _bf16(w_proj[:, rows].T),              # [256, 1024]
            "bqk_cols": bqk_cols,
            "bvT": _bf16(bv_l[None, :]),
            "C2": _bf16(C2),
            "O2": _bf16(O2),
            "ones512": _bf16(ones512),
        }
        if has_qgamma or has_kgamma:
            m["gamma_cols"] = gamma_cols
        if has_qbeta or has_kbeta:
            m["beta_cols"] = beta_cols
        in_maps.append(m)

    res = run_bass_kernel_spmd(nc, in_maps, core_ids=list(range(N_CORES)),
                               trace=TRACE)
    LAST_EXEC_NS = res.exec_time_ns
    globals()["LAST_RESULTS"] = res

    out = np.zeros((B, N, DIM), np.float32)
    for c in range(N_CORES):
        out[c // 4] += np.asarray(res.results[c]["out_partial"], np.float32)
    out += b_proj[None, None, :]
    return out
